# revision 13
# baseline (speedup 1.0000x reference)
"""Trainium2 Bass kernel for nn_CapsuleNetwork (BiLSTM + structured attention +
capsule dynamic routing), SPMD across 8 NeuronCores.

Sharding: direction x batch-quarter split. Core 2q runs the FORWARD LSTM for
batch quarter q (64 samples); core 2q+1 runs the BACKWARD LSTM on
host-reversed sequences of the same quarter. Each core keeps quarter-local
samples [0:32) (host orders each core's quarter as [kept | partner-kept]) for
the attention+routing phase; the partner's direction data for the kept
samples arrives via a pairwise ReduceScatter-add whose own-rank contribution
is zeroed by a per-core mask input. Direction asymmetries (sequence reversal,
own-first concat order) are folded into per-core INPUT data: per-sample
(T x T) selection/reversal matrices and row-reordered ws1/caps_w.

Packed-sequence masking is folded into the input projection: two extra input
rows (bias flag on valid steps, force flag on pads) map through augmented
weights to +/-30 gate pre-activations, so padded steps carry c exactly and
emit y=0 with no per-step mask ops. The program is fully input-generic.
"""
import sys

sys.path.insert(0, "/opt/trn_rl_repo")

import numpy as np
import ml_dtypes
from contextlib import ExitStack

import concourse.bass as bass
import concourse.tile as tile
from concourse import bacc, mybir

bf16 = ml_dtypes.bfloat16
F32 = mybir.dt.float32
BF16 = mybir.dt.bfloat16
AF = mybir.ActivationFunctionType
ALU = mybir.AluOpType
AX = mybir.AxisListType

# model dims
R, C, PCAP = 8, 64, 16
D, H, B, T, DA = 300, 512, 256, 64, 256
H2 = 2 * H                  # 1024
G = 4 * H                   # 2048 gate dim
GM = G // 128               # 16 gate m-tiles
KH = H // 128               # 4 h k-tiles
DAUG = 384                  # 302 used (D + bias + force) padded to 3 k-tiles
KD = DAUG // 128
Q = B // 4                  # 64 samples per core
S = Q // 2                  # 32 kept samples per core
NTOK = Q * T                # 4096 tokens (t-major: n = t*Q + s)
STOK = S * T                # 2048 attention tokens (s-major: n = s*T + t)
FORCE = 30.0
NCORE = 8

_CACHE = {}


def _build_nc():
    nc = bacc.Bacc("TRN2", target_bir_lowering=False, debug=False,
                   num_devices=NCORE)

    # ---- external inputs ----
    exT = nc.dram_tensor("exT", [DAUG, NTOK], BF16, kind="ExternalInput")
    whhT = nc.dram_tensor("whhT", [H, G], BF16, kind="ExternalInput")
    wauT = nc.dram_tensor("wauT", [DAUG, G], BF16, kind="ExternalInput")
    ws1T = nc.dram_tensor("ws1T", [H2, DA], BF16, kind="ExternalInput")
    ws2T = nc.dram_tensor("ws2T", [DA, R], BF16, kind="ExternalInput")
    capsw = nc.dram_tensor("capsw", [R, H2, C * PCAP], BF16, kind="ExternalInput")
    revm = nc.dram_tensor("revm", [2, S, T, T], BF16, kind="ExternalInput")
    mask01 = nc.dram_tensor("mask01", [2, 128, 1], F32, kind="ExternalInput")
    ident = nc.dram_tensor("ident", [128, 128], BF16, kind="ExternalInput")

    # ---- external outputs ----
    o_att = nc.dram_tensor("o_att", [S, R, T], F32, kind="ExternalOutput")
    o_cl = nc.dram_tensor("o_cl", [S, C], F32, kind="ExternalOutput")
    o_pred = nc.dram_tensor("o_pred", [S, R, C, PCAP], F32, kind="ExternalOutput")
    o_routes = nc.dram_tensor("o_routes", [S, R, C], F32, kind="ExternalOutput")

    # ---- internal DRAM ----
    xz_d = nc.dram_tensor("xz_d", [T, 128, GM, Q], BF16)       # [t, p, m, q]
    own_d = nc.dram_tensor("own_d", [T, H, S], BF16)           # own dir, kept
    send_d = nc.dram_tensor("send_d", [2, T, H, S], BF16)      # masked chunks
    recv_d = nc.dram_tensor("recv_d", [T, H, S], BF16)         # partner dir
    unrev_d = nc.dram_tensor("unrev_d", [2, STOK, H], BF16)    # (t,h) fixed up
    scd = nc.dram_tensor("scd", [S * R, T], F32)               # scores reorder

    with tile.TileContext(nc) as tc, ExitStack() as octx:
        wpool = octx.enter_context(tc.tile_pool(name="const", bufs=1))
        id_sb = wpool.tile([128, 128], BF16)
        nc.sync.dma_start(id_sb[:], ident[:])
        m01_sb = wpool.tile([128, 2], F32)
        nc.sync.dma_start(m01_sb[:], mask01[:].rearrange("c p one -> p (c one)"))
        semT = wpool.tile([128, 2 * KH, S, R], BF16)

        with ExitStack() as ctx:
            # ================= LSTM phase =================
            lpool = ctx.enter_context(tc.tile_pool(name="lstm", bufs=1))
            xzps = ctx.enter_context(tc.tile_pool(name="xzps", bufs=3, space="PSUM"))
            xzsb = ctx.enter_context(tc.tile_pool(name="xzsb", bufs=3))
            zps = ctx.enter_context(tc.tile_pool(name="zps", bufs=2, space="PSUM"))
            gpool = ctx.enter_context(tc.tile_pool(name="gates", bufs=2))
            xzld = ctx.enter_context(tc.tile_pool(name="xzld", bufs=3))

            whh_sb = lpool.tile([128, KH, G], BF16)
            nc.sync.dma_start(whh_sb[:],
                              whhT[:].rearrange("(k p) g -> p k g", p=128))
            wau_sb = lpool.tile([128, KD, G], BF16)
            nc.sync.dma_start(wau_sb[:],
                              wauT[:].rearrange("(k p) g -> p k g", p=128))
            exT_sb = lpool.tile([128, KD, NTOK], BF16)
            nc.sync.dma_start(exT_sb[:],
                              exT[:].rearrange("(k p) n -> p k n", p=128))

            # xz precompute: (G, NTOK) = WauT.T @ exT, streamed to DRAM t-major
            NCH = NTOK // 512           # 8 chunks = 8 timesteps each
            for m in range(GM):
                for cch in range(NCH):
                    ps = xzps.tile([128, 512], F32, tag="xzp")
                    for k in range(KD):
                        nc.tensor.matmul(
                            ps[:], wau_sb[:, k, m * 128:(m + 1) * 128],
                            exT_sb[:, k, cch * 512:(cch + 1) * 512],
                            start=(k == 0), stop=(k == KD - 1))
                    sb = xzsb.tile([128, 512], BF16, tag="xze")
                    nc.scalar.activation(sb[:], ps[:], AF.Copy)
                    nc.sync.dma_start(
                        xz_d[cch * 8:(cch + 1) * 8, :, m, :]
                        .rearrange("t p q -> p t q"),
                        sb[:].rearrange("p (t q) -> p t q", q=Q))

            # recurrence
            y_sb = lpool.tile([128, T + 1, KH, Q], BF16)
            c_sb = lpool.tile([128, KH, Q], F32)
            nc.vector.memset(y_sb[:, 0], 0.0)
            nc.vector.memset(c_sb[:], 0.0)

            for t in range(T):
                xz_t = xzld.tile([128, GM, Q], BF16, tag="xzt")
                nc.sync.dma_start(xz_t[:], xz_d[t])
                zp = zps.tile([128, GM, Q], F32, tag="z")
                for m in range(GM):
                    for k in range(KH):
                        nc.tensor.matmul(
                            zp[:, m], whh_sb[:, k, m * 128:(m + 1) * 128],
                            y_sb[:, t, k], start=(k == 0), stop=(k == KH - 1))
                z_sb = gpool.tile([128, GM, Q], BF16, tag="zsb")
                for grp in range(4):
                    nc.vector.tensor_add(
                        z_sb[:, grp * 4:(grp + 1) * 4],
                        zp[:, grp * 4:(grp + 1) * 4],
                        xz_t[:, grp * 4:(grp + 1) * 4])
                # gate order (torch): m 0:4=i, 4:8=f, 8:12=g, 12:16=o
                sig_if = gpool.tile([128, 8, Q], BF16, tag="sif")
                nc.scalar.activation(sig_if[:], z_sb[:, 0:8], AF.Sigmoid)
                tg = gpool.tile([128, KH, Q], BF16, tag="tg")
                nc.scalar.activation(tg[:], z_sb[:, 8:12], AF.Tanh)
                so = gpool.tile([128, KH, Q], BF16, tag="so")
                nc.scalar.activation(so[:], z_sb[:, 12:16], AF.Sigmoid)
                t1 = gpool.tile([128, KH, Q], BF16, tag="t1")
                nc.vector.tensor_mul(t1[:], sig_if[:, 0:4], tg[:])
                nc.vector.tensor_mul(c_sb[:], sig_if[:, 4:8], c_sb[:])
                nc.vector.tensor_add(c_sb[:], c_sb[:], t1[:])
                tc_t = gpool.tile([128, KH, Q], BF16, tag="tct")
                nc.scalar.activation(tc_t[:], c_sb[:], AF.Tanh)
                nc.vector.tensor_mul(y_sb[:, t + 1], so[:], tc_t[:])

            # own kept half -> own_d (t, h, s); one 3-dim DMA per timestep
            for t in range(T):
                nc.sync.dma_start(
                    own_d[t].rearrange("(k p) s -> p k s", p=128),
                    y_sb[:, t + 1, :, 0:S])
            # partner-destined half, masked into both RS chunks
            send_sb = lpool.tile([128, 2, T, KH, S], BF16)
            nc.scalar.activation(send_sb[:, 0], y_sb[:, 1:, :, S:Q], AF.Copy,
                                 scale=m01_sb[:, 0:1])
            nc.scalar.activation(send_sb[:, 1], y_sb[:, 1:, :, S:Q], AF.Copy,
                                 scale=m01_sb[:, 1:2])
            for ch in range(2):
                for t in range(T):
                    nc.sync.dma_start(
                        send_d[ch, t].rearrange("(k p) s -> p k s", p=128),
                        send_sb[:, ch, t])

        # ================= exchange =================
        nc.gpsimd.collective_compute(
            "ReduceScatter", ALU.add,
            replica_groups=[[0, 1], [2, 3], [4, 5], [6, 7]],
            ins=[send_d[:]], outs=[recv_d[:]])

        with ExitStack() as actx:
            apool = actx.enter_context(tc.tile_pool(name="attn", bufs=1))
            asb = actx.enter_context(tc.tile_pool(name="asb", bufs=2))

            # ---- fixup: (t,h) layout + unreversal/selection ----
            raw_own = apool.tile([T, H, S], BF16)
            nc.sync.dma_start(raw_own[:], own_d[:])
            raw_par = apool.tile([T, H, S], BF16)
            nc.sync.dma_start(raw_par[:], recv_d[:])
            rev_sb = apool.tile([T, 2, S, T], BF16)
            nc.sync.dma_start(rev_sb[:], revm[:].rearrange("c s a b -> a c s b"))

            out_bt = apool.tile([T, S, H2], BF16)
            with ExitStack() as pctx:
                fps = pctx.enter_context(
                    tc.tile_pool(name="fps", bufs=4, space="PSUM"))
                for half, raw in ((0, raw_own), (1, raw_par)):
                    for j in range(S):
                        psf = fps.tile([T, H], F32, tag="fix")
                        nc.tensor.matmul(psf[:], rev_sb[:, half, j],
                                         raw[:, :, j], start=True, stop=True)
                        dst = out_bt[:, j, half * H:(half + 1) * H]
                        if j % 2 == 0:
                            nc.scalar.activation(dst, psf[:], AF.Copy)
                        else:
                            nc.vector.tensor_copy(dst, psf[:])
                    nc.sync.dma_start(
                        unrev_d[half].rearrange("(s t) h -> t s h", t=T),
                        out_bt[:, :, half * H:(half + 1) * H])

            outT_sb = apool.tile([128, 2 * KH, STOK], BF16)
            nc.sync.dma_start_transpose(outT_sb[:, 0:KH], unrev_d[0])
            nc.sync.dma_start_transpose(outT_sb[:, KH:2 * KH], unrev_d[1])

            # ---- s1 = tanh(ws1 @ out), scores = ws2 @ s1 ----
            ws1_sb = apool.tile([128, 2 * KH, DA], BF16)
            nc.sync.dma_start(ws1_sb[:],
                              ws1T[:].rearrange("(k p) a -> p k a", p=128))
            ws2_sb = apool.tile([128, 2, R], BF16)
            nc.sync.dma_start(ws2_sb[:],
                              ws2T[:].rearrange("(k p) r -> p k r", p=128))

            s1_sb = apool.tile([128, 2, STOK], BF16)
            sc_sb = apool.tile([R, STOK], F32)
            with ExitStack() as pctx:
                mps = pctx.enter_context(
                    tc.tile_pool(name="mps", bufs=2, space="PSUM"))
                for m in range(2):
                    for cch in range(STOK // 512):
                        ps = mps.tile([128, 512], F32, tag="s1p")
                        for k in range(2 * KH):
                            nc.tensor.matmul(
                                ps[:], ws1_sb[:, k, m * 128:(m + 1) * 128],
                                outT_sb[:, k, cch * 512:(cch + 1) * 512],
                                start=(k == 0), stop=(k == 2 * KH - 1))
                        nc.scalar.activation(
                            s1_sb[:, m, cch * 512:(cch + 1) * 512], ps[:],
                            AF.Tanh)
                for cch in range(STOK // 512):
                    ps = mps.tile([R, 512], F32, tag="scp")
                    for k in range(2):
                        nc.tensor.matmul(
                            ps[:], ws2_sb[:, k, :],
                            s1_sb[:, k, cch * 512:(cch + 1) * 512],
                            start=(k == 0), stop=(k == 1))
                    nc.vector.tensor_copy(sc_sb[:, cch * 512:(cch + 1) * 512],
                                          ps[:])

            # reorder scores via DRAM to partition = s*R + r
            nc.sync.dma_start(
                scd[:].rearrange("(s r) t -> r s t", r=R),
                sc_sb[:].rearrange("r (s t) -> r s t", t=T))
            attp = apool.tile([128, 2, T], F32)
            nc.sync.dma_start(attp[:],
                              scd[:].rearrange("(i p) t -> p i t", p=128))

            # ---- softmax over t ----
            mx = asb.tile([128, 2], F32, tag="mx")
            esum = asb.tile([128, 2], F32, tag="es")
            einv = asb.tile([128, 2], F32, tag="ei")
            for i in range(2):
                nc.vector.tensor_reduce(mx[:, i:i + 1], attp[:, i],
                                        axis=AX.X, op=ALU.max)
                nc.vector.tensor_scalar_sub(attp[:, i], attp[:, i],
                                            mx[:, i:i + 1])
                nc.scalar.activation(attp[:, i], attp[:, i], AF.Exp,
                                     accum_out=esum[:, i:i + 1])
                nc.vector.reciprocal(einv[:, i:i + 1], esum[:, i:i + 1])
                nc.vector.tensor_scalar_mul(attp[:, i], attp[:, i],
                                            einv[:, i:i + 1])
            # o_att[(i*16+sp), r, t] = attp[(sp*8+r), i, t]
            nc.sync.dma_start(
                o_att[:].rearrange("(i sp) r t -> sp r i t", sp=16), attp[:])
            attb = apool.tile([128, 2, T], BF16)
            nc.vector.tensor_copy(attb[:], attp[:])

            # ---- attT via PE transpose, then sem ----
            attT = apool.tile([T, 2, 128], BF16)
            with ExitStack() as pctx:
                sps = pctx.enter_context(
                    tc.tile_pool(name="sps", bufs=2, space="PSUM"))
                for i in range(2):
                    pst = sps.tile([T, 128], BF16, tag="atp")
                    nc.tensor.transpose(pst[:], attb[:, i, :], id_sb[:])
                    nc.scalar.activation(attT[:, i, :], pst[:], AF.Copy)
                for s in range(S):
                    pss = sps.tile([128, 2 * KH, R], F32, tag="semp")
                    for m in range(2 * KH):
                        nc.tensor.matmul(
                            pss[:, m], out_bt[:, s, m * 128:(m + 1) * 128],
                            attT[:, s // 16, (s % 16) * R:(s % 16 + 1) * R],
                            start=True, stop=True)
                    nc.vector.tensor_copy(semT[:, :, s, :], pss[:])

        with ExitStack() as cctx:
            # ============ pred + routing ============
            cpool = cctx.enter_context(tc.tile_pool(name="caps", bufs=2))
            rpool = cctx.enter_context(tc.tile_pool(name="rout", bufs=1))
            rps = cctx.enter_context(tc.tile_pool(name="rps", bufs=2, space="PSUM"))
            rsb = cctx.enter_context(tc.tile_pool(name="rsb", bufs=1))

            pred_cpr = rpool.tile([S, C, PCAP, R], BF16)
            pred_rcp = rpool.tile([S, R, C, PCAP], BF16)
            for r in range(R):
                capsb = cpool.tile([128, 2 * KH, C * PCAP], BF16, tag="caps")
                nc.sync.dma_start(
                    capsb[:], capsw[r].rearrange("(k p) n -> p k n", p=128))
                psp = rps.tile([S, C * PCAP], F32, tag="predp")
                for n in range(2):
                    for k in range(2 * KH):
                        nc.tensor.matmul(
                            psp[:, n * 512:(n + 1) * 512],
                            semT[:, k, :, r],
                            capsb[:, k, n * 512:(n + 1) * 512],
                            start=(k == 0), stop=(k == 2 * KH - 1))
                stg = rsb.tile([S, C * PCAP], F32, tag="stg")
                nc.scalar.activation(stg[:], psp[:], AF.Copy)
                nc.sync.dma_start(
                    o_pred[:, r], stg[:].rearrange("s (c p) -> s c p", p=PCAP))
                nc.vector.tensor_copy(
                    pred_cpr[:, :, :, r],
                    psp[:].rearrange("s (c p) -> s c p", p=PCAP))
                nc.scalar.activation(
                    pred_rcp[:, r],
                    psp[:].rearrange("s (c p) -> s c p", p=PCAP), AF.Copy)

            logits = rpool.tile([S, R, C], F32)
            routes = rpool.tile([S, R, C], F32)
            vsq = rpool.tile([S, C, PCAP], F32)
            sq = rpool.tile([S, C], F32)
            eps_t = rpool.tile([S, 1], F32)
            nc.vector.memset(eps_t[:], 1e-9)
            nc.vector.memset(logits[:], 0.0)

            for it in range(3):
                if it == 0:
                    nc.vector.memset(routes[:], 1.0 / C)
                else:
                    rmx = rsb.tile([S, R], F32, tag="rmx")
                    nc.vector.tensor_reduce(rmx[:], logits[:], axis=AX.X,
                                            op=ALU.max)
                    nc.vector.tensor_tensor(
                        out=routes[:], in0=logits[:],
                        in1=rmx[:].unsqueeze(2).to_broadcast([S, R, C]),
                        op=ALU.subtract)
                    nc.scalar.activation(routes[:], routes[:], AF.Exp)
                    rsum = rsb.tile([S, R], F32, tag="rsum")
                    nc.vector.tensor_reduce(rsum[:], routes[:], axis=AX.X,
                                            op=ALU.add)
                    rinv = rsb.tile([S, R], F32, tag="rinv")
                    nc.vector.reciprocal(rinv[:], rsum[:])
                    nc.vector.tensor_tensor(
                        out=routes[:], in0=routes[:],
                        in1=rinv[:].unsqueeze(2).to_broadcast([S, R, C]),
                        op=ALU.mult)
                if it == 2:
                    nc.sync.dma_start(o_routes[:], routes[:])
                # v = sum_r routes * pred   (c, p, r layout; reduce innermost)
                prod = rsb.tile([S, C, PCAP, R], F32, tag="prod")
                nc.vector.tensor_tensor(
                    out=prod[:], in0=pred_cpr[:],
                    in1=routes[:].transpose([0, 2, 1]).unsqueeze(2)
                    .to_broadcast([S, C, PCAP, R]),
                    op=ALU.mult)
                vraw = rsb.tile([S, C, PCAP], F32, tag="vraw")
                nc.vector.tensor_reduce(
                    vraw[:], prod[:].rearrange("s c p r -> s (c p) r"),
                    axis=AX.X, op=ALU.add)
                # squash
                v2 = rsb.tile([S, C, PCAP], F32, tag="v2")
                nc.vector.tensor_mul(v2[:], vraw[:], vraw[:])
                nc.vector.tensor_reduce(sq[:], v2[:], axis=AX.X, op=ALU.add)
                st = rsb.tile([S, C], F32, tag="st")
                nc.scalar.activation(st[:], sq[:], AF.Sqrt, bias=eps_t[:])
                sp1 = rsb.tile([S, C], F32, tag="sp1")
                nc.vector.tensor_scalar_add(sp1[:], sq[:], 1.0)
                nc.vector.tensor_mul(sp1[:], sp1[:], st[:])
                f2 = rsb.tile([S, C], F32, tag="f2")
                nc.vector.reciprocal(f2[:], sp1[:])
                nc.vector.tensor_mul(f2[:], f2[:], sq[:])
                if it == 2:
                    # class_logits = |squash(v)| = f2 * sqrt(sq)
                    st2 = rsb.tile([S, C], F32, tag="st2")
                    nc.scalar.activation(st2[:], sq[:], AF.Sqrt)
                    cl = rsb.tile([S, C], F32, tag="cl")
                    nc.vector.tensor_mul(cl[:], f2[:], st2[:])
                    nc.sync.dma_start(o_cl[:], cl[:])
                else:
                    nc.vector.tensor_tensor(
                        out=vsq[:], in0=vraw[:],
                        in1=f2[:].unsqueeze(2).to_broadcast([S, C, PCAP]),
                        op=ALU.mult)
                    # agreement: logits += sum_p pred * vsq
                    prod2 = rsb.tile([S, R, C, PCAP], F32, tag="prod2")
                    nc.vector.tensor_tensor(
                        out=prod2[:], in0=pred_rcp[:],
                        in1=vsq[:].unsqueeze(1).to_broadcast([S, R, C, PCAP]),
                        op=ALU.mult)
                    ag = rsb.tile([S, R, C], F32, tag="ag")
                    nc.vector.tensor_reduce(
                        ag[:], prod2[:].rearrange("s r c p -> s (r c) p"),
                        axis=AX.X, op=ALU.add)
                    nc.vector.tensor_add(logits[:], logits[:], ag[:])

    nc.compile()
    return nc


def _get_nc():
    if "nc" not in _CACHE:
        _CACHE["nc"] = _build_nc()
    return _CACHE["nc"]


# ======================= host side =======================

def _make_wau(wih, bih, bhh):
    wau = np.zeros((DAUG, G), np.float32)
    wau[:D] = wih.T.astype(np.float32)
    wau[D] = (bih + bhh).astype(np.float32)
    fv = np.zeros(G, np.float32)
    fv[0 * H:1 * H] = -FORCE   # i -> 0
    fv[1 * H:2 * H] = +FORCE   # f -> 1
    fv[2 * H:3 * H] = 0.0      # g (tanh(0)=0)
    fv[3 * H:4 * H] = -FORCE   # o -> 0 (y=0 at pads)
    wau[D + 1] = fv
    return wau


def _prep_core_inputs(idx, direction, ex_all, lens, whh, wau_T, ws1_own,
                      caps_own, ws2T_b):
    lens_c = lens[idx].astype(np.int64)
    ex = ex_all[idx]                                     # (Q, T, D)
    tpos = np.arange(T)
    maskc = tpos[None, :] < lens_c[:, None]              # (Q, T)
    if direction == 1:
        rev = np.where(maskc, lens_c[:, None] - 1 - tpos[None, :],
                       tpos[None, :])
        ex = np.take_along_axis(ex, rev[:, :, None], axis=1)

    exa = np.zeros((Q, T, DAUG), np.float32)
    exa[:, :, :D] = np.where(maskc[:, :, None], ex, 0.0)
    exa[:, :, D] = maskc.astype(np.float32)              # bias flag
    exa[:, :, D + 1] = (~maskc).astype(np.float32)       # force flag
    exTv = np.ascontiguousarray(exa.transpose(2, 1, 0)   # (DAUG, T, Q)
                                ).reshape(DAUG, NTOK).astype(bf16)

    # selection/reversal matrices for kept samples [0:S)
    revmv = np.zeros((2, S, T, T), np.float32)
    for half in range(2):
        is_rev = (direction == 1) if half == 0 else (direction == 0)
        for j in range(S):
            m = np.eye(T, dtype=np.float32)
            if is_rev:
                L = int(lens_c[j])
                m[:L, :L] = 0.0
                m[np.arange(L - 1, -1, -1), np.arange(L)] = 1.0
            revmv[half, j] = m

    m01 = np.zeros((2, 128, 1), np.float32)
    m01[1 - direction, :, :] = 1.0   # zero own-rank chunk, fill partner's

    return dict(
        exT=exTv,
        whhT=np.ascontiguousarray(whh.T).astype(bf16),
        wauT=wau_T.astype(bf16),
        ws1T=np.ascontiguousarray(ws1_own).astype(bf16),
        ws2T=ws2T_b,
        capsw=np.ascontiguousarray(caps_own).astype(bf16),
        revm=revmv.astype(bf16),
        mask01=m01,
        ident=np.eye(128, dtype=np.float32).astype(bf16),
    )


def _run(inputs, trace=False):
    from concourse.bass_utils import run_bass_kernel_spmd

    x = np.asarray(inputs["input_x"])
    lens = np.asarray(inputs["lens"]).astype(np.int64)
    emb = np.asarray(inputs["emb"], np.float32)
    ex_all = emb[x]                                      # (B, T, D)

    wau_f = _make_wau(np.asarray(inputs["wih_f"], np.float32),
                      np.asarray(inputs["bih_f"], np.float32),
                      np.asarray(inputs["bhh_f"], np.float32))
    wau_b = _make_wau(np.asarray(inputs["wih_b"], np.float32),
                      np.asarray(inputs["bih_b"], np.float32),
                      np.asarray(inputs["bhh_b"], np.float32))
    whh_f = np.asarray(inputs["whh_f"], np.float32)
    whh_b = np.asarray(inputs["whh_b"], np.float32)
    ws1 = np.asarray(inputs["ws1"], np.float32)          # (DA, 2H)
    ws2 = np.asarray(inputs["ws2"], np.float32)          # (R, DA)
    caps = np.asarray(inputs["caps_w"], np.float32)      # (R, 2H, C*P)

    ws1T_f = np.ascontiguousarray(ws1.T)                 # fwd rows first
    ws1T_b = np.concatenate([ws1.T[H:], ws1.T[:H]], axis=0)
    caps_f = caps
    caps_b = np.concatenate([caps[:, H:], caps[:, :H]], axis=1)
    ws2T_b16 = np.ascontiguousarray(ws2.T).astype(bf16)

    in_maps = []
    core_keep = []
    for q in range(4):
        base = np.arange(B)[q::4]
        order_even = base
        order_odd = np.concatenate([base[S:], base[:S]])
        in_maps.append(_prep_core_inputs(order_even, 0, ex_all, lens, whh_f,
                                         wau_f, ws1T_f, caps_f, ws2T_b16))
        core_keep.append(order_even[:S])
        in_maps.append(_prep_core_inputs(order_odd, 1, ex_all, lens, whh_b,
                                         wau_b, ws1T_b, caps_b, ws2T_b16))
        core_keep.append(order_odd[:S])

    nc = _get_nc()
    res = run_bass_kernel_spmd(nc, in_maps, list(range(NCORE)), trace=trace)

    att = np.zeros((B, R, T), np.float32)
    cl = np.zeros((B, C), np.float32)
    pred = np.zeros((B, R, C, PCAP), np.float32)
    routes = np.zeros((B, R, C), np.float32)
    for c in range(NCORE):
        r = res.results[c]
        gidx = core_keep[c]
        att[gidx] = r["o_att"]
        cl[gidx] = r["o_cl"]
        pred[gidx] = r["o_pred"]
        routes[gidx] = r["o_routes"]
    return (att, cl, pred, routes), res


def kernel(**inputs):
    outs, _ = _run(inputs, trace=False)
    return outs


def timed_run(inputs, iters=20):
    """Steady-state per-execution wall time of the NEFF across 8 cores.

    Mirrors bass2jax.run_bass_via_pjrt but keeps inputs device-resident and
    dispatches `iters` executions back-to-back (async) so the per-exec time
    approaches pure device execution time. Returns (outputs, ns_per_exec).
    """
    import time
    import jax
    from jax.experimental.shard_map import shard_map
    from jax.sharding import Mesh, PartitionSpec
    from concourse import bass2jax, mybir as mb

    in_maps = _prep_all(inputs)
    nc = _get_nc()
    bass2jax.install_neuronx_cc_hook()

    partition_name = (nc.partition_id_tensor.name
                      if nc.partition_id_tensor else None)
    in_names, out_names, out_avals, zero_outs = [], [], [], []
    for alloc in nc.m.functions[0].allocations:
        if not isinstance(alloc, mb.MemoryLocationSet):
            continue
        name = alloc.memorylocations[0].name
        if alloc.kind == "ExternalInput":
            if name != partition_name:
                in_names.append(name)
        elif alloc.kind == "ExternalOutput":
            shape = tuple(alloc.tensor_shape)
            dtype = mb.dt.np(alloc.dtype)
            out_names.append(name)
            out_avals.append(jax.core.ShapedArray(shape, dtype))
            zero_outs.append(np.zeros(shape, dtype))
    n_params = len(in_names)
    all_in_names = in_names + out_names + (
        [partition_name] if partition_name else [])

    def _body(*args):
        operands = list(args)
        if partition_name is not None:
            operands.append(bass2jax.partition_id_tensor())
        outs = bass2jax._bass_exec_p.bind(
            *operands,
            out_avals=tuple(out_avals),
            in_names=tuple(all_in_names),
            out_names=tuple(out_names),
            lowering_input_output_aliases=(),
            sim_require_finite=True,
            sim_require_nnan=True,
            nc=nc,
        )
        return tuple(outs)

    devices = jax.devices()[:NCORE]
    mesh = Mesh(np.asarray(devices), ("core",))
    n_outs = len(out_names)
    sharded = jax.jit(
        shard_map(_body, mesh=mesh,
                  in_specs=(PartitionSpec("core"),) * (n_params + n_outs),
                  out_specs=(PartitionSpec("core"),) * n_outs,
                  check_rep=False),
        keep_unused=True,
    )
    concat_in = [
        np.concatenate([np.asarray(in_maps[c][nm]) for c in range(NCORE)],
                       axis=0)
        for nm in in_names
    ]
    concat_zero = [np.concatenate([z] * NCORE, axis=0) for z in zero_outs]
    sh = jax.sharding.NamedSharding(mesh, PartitionSpec("core"))
    dev_in = [jax.device_put(a, sh) for a in concat_in]
    dev_zero = [jax.device_put(a, sh) for a in concat_zero]

    # warmup (compile + first exec)
    outs = sharded(*dev_in, *dev_zero)
    jax.block_until_ready(outs)

    t0 = time.perf_counter()
    outs1 = sharded(*dev_in, *dev_zero)
    jax.block_until_ready(outs1)
    t1 = time.perf_counter()
    single = t1 - t0

    t0 = time.perf_counter()
    last = None
    for _ in range(iters):
        last = sharded(*dev_in, *dev_zero)
    jax.block_until_ready(last)
    t1 = time.perf_counter()
    per_exec = (t1 - t0) / iters
    ns = per_exec * 1e9
    return dict(single_ns=single * 1e9, pipelined_ns=ns)


def _prep_all(inputs):
    """Host prep shared by _run and timed_run: per-core input maps."""
    x = np.asarray(inputs["input_x"])
    lens = np.asarray(inputs["lens"]).astype(np.int64)
    emb = np.asarray(inputs["emb"], np.float32)
    ex_all = emb[x]

    wau_f = _make_wau(np.asarray(inputs["wih_f"], np.float32),
                      np.asarray(inputs["bih_f"], np.float32),
                      np.asarray(inputs["bhh_f"], np.float32))
    wau_b = _make_wau(np.asarray(inputs["wih_b"], np.float32),
                      np.asarray(inputs["bih_b"], np.float32),
                      np.asarray(inputs["bhh_b"], np.float32))
    whh_f = np.asarray(inputs["whh_f"], np.float32)
    whh_b = np.asarray(inputs["whh_b"], np.float32)
    ws1 = np.asarray(inputs["ws1"], np.float32)
    ws2 = np.asarray(inputs["ws2"], np.float32)
    caps = np.asarray(inputs["caps_w"], np.float32)

    ws1T_f = np.ascontiguousarray(ws1.T)
    ws1T_b = np.concatenate([ws1.T[H:], ws1.T[:H]], axis=0)
    caps_f = caps
    caps_b = np.concatenate([caps[:, H:], caps[:, :H]], axis=1)
    ws2T_b16 = np.ascontiguousarray(ws2.T).astype(bf16)

    in_maps = []
    for q in range(4):
        base = np.arange(B)[q::4]
        order_even = base
        order_odd = np.concatenate([base[S:], base[:S]])
        in_maps.append(_prep_core_inputs(order_even, 0, ex_all, lens, whh_f,
                                         wau_f, ws1T_f, caps_f, ws2T_b16))
        in_maps.append(_prep_core_inputs(order_odd, 1, ex_all, lens, whh_b,
                                         wau_b, ws1T_b, caps_b, ws2T_b16))
    return in_maps
